# revision 51
# baseline (speedup 1.0000x reference)
"""AutoCorrelation block (FFT cross-correlation attention) on 8 Trainium2 cores.

Math (per batch b, faithfully reproducing the reference):
  qh = q @ Wq + bq, kh = k @ Wk + bk         (v projection is dead code)
  per channel c=(h,dh) (512 per batch):
    r = irfft(rfft(qh_c) * conj(rfft(kh_c)))   # circular cross-correlation
    top-8 lags d_k of r, softmax of the 8 values -> w_k
    agg_c[t] = sum_k w_k * qh_c[(t + d_k) % L]
  out = agg^T @ Wo + bo

Implementation: DFT-as-matmul with a stacked real cos/sin basis (shared by all
channels, so the whole FFT pipeline is dense PE work). All matmul operands are
fp16 (fp32 PSUM accumulation): fp16 moving operands run at 1 cycle/row and the
quantization error (~5e-3 on the final output) is well inside the 2e-2 gate.
Key structure:
  - projections with quarter-tile-paced input DMA (PE starts after ~0.8 MB);
  - channel-major qh (gather source) via PE transposes of the fp16 htd tiles
    with the bias fused into the DVE PSUM-drain (per-partition add);
  - forward DFT with the frequency product computed inline (Z never leaves
    SBUF, fp16, 2/L irfft scale folded into the kf copy so the inverse basis
    is unscaled cos/sin — exact in fp16 and fully SBUF-resident at 8 MB);
  - inverse DFT interleaved per 128-channel chunk with top-8 (DVE max/
    max_index), softmax, diag(w) stationary construction, indirect-DMA
    gathers (4 SWDGE queues) and the weighted sum as PE matmuls with
    diag(w_k) stationaries — gathers of chunk i overlap inv-DFT of chunk i+1;
  - output projection computed TRANSPOSED (channel-major) so bo is a
    per-partition bias fused into the PSUM copy; the host un-transposes.

Sharding: data-parallel over batch. B == 8 == n_cores, one batch per core,
weights + DFT matrices replicated. No collectives.
"""

import numpy as np

import concourse.bass as bass
import concourse.bacc as bacc
import concourse.mybir as mybir
import concourse.tile as tile
from concourse.bass import IndirectOffsetOnAxis, ts
from concourse.bass_utils import run_bass_kernel_spmd

B, L, D = 8, 2048, 512
TOPK = 8
NF = 1025          # rfft bins for L=2048
FS = 2048          # stacked freq rows: 16 chunks of 128
IM0 = 1024         # sin(f) block at 1024+f (f=1..1023); slot 1024 = Nyquist cos
N_CORES = 8
KC = 4             # d_in chunks of 128
TM = 16            # time chunks of 128
CN = 4             # channel chunks of 128
FM = 16            # stacked-freq chunks of 128

F32 = mybir.dt.float32
F32R = mybir.dt.float32r
U32 = mybir.dt.uint32
BF16 = mybir.dt.bfloat16
F16 = mybir.dt.float16
AF = mybir.ActivationFunctionType
AX = mybir.AxisListType


def _build_dft_mats():
    # Two-level DIT split: all folds are partition-aligned tile adds, all
    # twiddles absorbed into three branch-specific real bases.
    #   b1: odd bins f=2i+1       over xo  = x[:1024] - x[1024:]      (8 chunks)
    #   b2: f=2h, h odd           over xeo = fold2(xe) minus          (4 chunks)
    #   b3: f=4g (incl DC/Nyq)    over xee = fold2(xe) plus           (4 chunks)
    # Chunk-pair stacking (re, im) mirrors the original full basis; b3 keeps
    # the DC row and stores the Nyquist cos row in the sin(0) slot.
    t1 = np.arange(1024)
    t2 = np.arange(512)
    h1 = 2 * np.arange(512) + 1
    B1 = np.zeros((1024, 1024), np.float32)
    a1 = (2.0 * np.pi / 2048.0) * (np.outer(t1, h1) % 2048)
    B1[:, :512] = np.cos(a1)
    B1[:, 512:] = np.sin(a1)
    h2 = 2 * np.arange(256) + 1
    B2 = np.zeros((512, 512), np.float32)
    a2 = (2.0 * np.pi / 1024.0) * (np.outer(t2, h2) % 1024)
    B2[:, :256] = np.cos(a2)
    B2[:, 256:] = np.sin(a2)
    g3 = np.arange(256)
    B3 = np.zeros((512, 512), np.float32)
    a3 = (2.0 * np.pi / 512.0) * (np.outer(t2, g3) % 512)
    B3[:, :256] = np.cos(a3)
    B3[:, 256] = np.where(t2 % 2 == 0, 1.0, -1.0)
    B3[:, 257:] = np.sin(a3[:, 1:])
    return B1, B2, B3


def _kernel_body(tc, dr, out_ap, q2):
    nc = tc.nc

    w_pool = tc.alloc_tile_pool(name="weights", bufs=1)
    cf_pool = tc.alloc_tile_pool(name="cf", bufs=4, side="right")
    htd_pool = tc.alloc_tile_pool(name="htd", bufs=1, side="right")

    # ---- S1 inputs first so the PE can start ASAP ----
    qt_pool = tc.alloc_tile_pool(name="qt", bufs=1)
    qt = [qt_pool.tile([128, L], F16, tag=f"qt{i}", name=f"qt{i}") for i in range(KC)]
    kt = [qt_pool.tile([128, L], F16, tag=f"kt{i}", name=f"kt{i}") for i in range(KC)]

    # ---- constants (DMA order matters: the sync queue is in-order, so load
    # exactly what the first matmul group needs first) ----
    wqk_pool = tc.alloc_tile_pool(name="wqk", bufs=1)
    wq_t = wqk_pool.tile([128, KC * D], F16, tag="wqt", name="wqt")
    wk_t = wqk_pool.tile([128, KC * D], F16, tag="wkt", name="wkt")
    wo_t = w_pool.tile([128, KC * D], F16, tag="wot", name="wot")
    # tiny constants first (37 KB): bias rows and ident must not sit behind
    # megabyte loads — the grp-0 bias matmul needs them at ~14us.
    ident = w_pool.tile([128, 128], F16, tag="ident", name="ident")
    nc.sync.dma_start(ident[:, :], dr["ident"][:, :])
    brow = {}
    for nm in ("bqL", "bkL2"):
        brow[nm] = w_pool.tile([1, D], F16, tag=f"{nm}r", name=f"{nm}r")
        nc.sync.dma_start(brow[nm][:, :], dr[nm][:, :])
    bqcol = w_pool.tile([128, CN], F32, tag="bqc", name="bqc")
    for c in range(CN):
        nc.sync.dma_start(bqcol[:, c : c + 1], dr["bqc"][ts(c, 128), :])
    bocol = w_pool.tile([128, CN], F32, tag="boc", name="boc")
    for c in range(CN):
        nc.sync.dma_start(bocol[:, c : c + 1], dr["boc"][ts(c, 128), :])
    # quarter-tile interleaved loads: the first matmul group needs ~0.8 MB
    for i in range(KC):
        nc.sync.dma_start(qt[i][:, 0:512], dr["qT"][ts(i, 128), 0:512])
        nc.sync.dma_start(wq_t[:, ts(i, D)], dr["Wq"][:, ts(i, D)])
    for q4 in range(1, 4):
        for i in range(KC):
            nc.sync.dma_start(qt[i][:, ts(q4, 512)], dr["qT"][ts(i, 128), ts(q4, 512)])
    # k-side after q-side on the same queue: full bandwidth for the critical
    # path, and kt still lands well before the kh projection (~31us)
    for i in range(KC):
        nc.sync.dma_start(wk_t[:, ts(i, D)], dr["Wk"][:, ts(i, D)])
        nc.sync.dma_start(kt[i][:, 0:512], dr["kT"][ts(i, 128), 0:512])
    for q4 in range(1, 4):
        for i in range(KC):
            nc.sync.dma_start(kt[i][:, ts(q4, 512)], dr["kT"][ts(i, 128), ts(q4, 512)])
    nc.scalar.dma_start(wo_t[:, :], dr["Wo"][:, :])
    wq = [wq_t[:, ts(i, D)] for i in range(KC)]
    wk = [wk_t[:, ts(i, D)] for i in range(KC)]
    wo = [wo_t[:, ts(i, D)] for i in range(KC)]

    htd_q = [htd_pool.tile([128, D], F16, tag=f"hq{m}", name=f"hq{m}") for m in range(TM)]
    fo_k = [htd_pool.tile([128, D], F16, tag=f"fok{m}", name=f"fok{m}") for m in range(8)]
    feo_k = [htd_pool.tile([128, D], F16, tag=f"eok{m}", name=f"eok{m}") for m in range(4)]
    fee_k = [htd_pool.tile([128, D], F16, tag=f"eek{m}", name=f"eek{m}") for m in range(4)]
    ktgt = fo_k + feo_k + fee_k

    # ---- S1/S2: projections (all-fp16 operands, fp32 PSUM accumulate) ----
    ps1 = tc.alloc_tile_pool(name="ps1", bufs=6, space="PSUM")
    qht_pool = tc.alloc_tile_pool(name="qht", bufs=2)

    # qh_td[t, c] = sum_di qT[di, t] * Wq[di, c]; bias is applied in the
    # channel-major transpose copies (per-partition there) and via the DC-bin
    # fix in the forward DFT.
    for grp in range(4):
        pss1 = [ps1.tile([128, D], F32, tag="p1", name="p1") for _ in range(4)]
        for kc in range(KC):
            for m4 in range(4):
                nc.tensor.matmul(
                    pss1[m4][:, :], qt[kc][:, ts(grp * 4 + m4, 128)], wq[kc],
                    start=(kc == 0), stop=(kc == KC - 1),
                )
        for m4 in range(4):
            nc.scalar.activation(
                htd_q[grp * 4 + m4][:, :], pss1[m4][:, :], AF.Copy
            )
    # ---- DIT folds: partition-aligned tile adds (t and t+1024 share the
    # partition), spread over DVE and gpsimd. fo feeds the odd-bin branch,
    # fee/feo the two level-2 branches.
    fo_q = [w_pool.tile([128, D], F16, tag=f"foq{m}", name=f"foq{m}") for m in range(8)]
    fee_q = [w_pool.tile([128, D], F16, tag=f"eeq{m}", name=f"eeq{m}") for m in range(4)]
    feo_q = [w_pool.tile([128, D], F16, tag=f"eoq{m}", name=f"eoq{m}") for m in range(4)]
    for m in range(8):
        eng = nc.vector if m % 2 == 0 else nc.gpsimd
        eng.tensor_sub(fo_q[m][:, :], htd_q[m][:, :], htd_q[m + 8][:, :])
    for m in range(4):
        pa = w_pool.tile([128, D], F16, tag="pa", name="pa")
        pb = w_pool.tile([128, D], F16, tag="pb", name="pb")
        nc.vector.tensor_add(pa[:, :], htd_q[m][:, :], htd_q[m + 8][:, :])
        nc.gpsimd.tensor_add(pb[:, :], htd_q[m + 4][:, :], htd_q[m + 12][:, :])
        nc.vector.tensor_add(fee_q[m][:, :], pa[:, :], pb[:, :])
        nc.gpsimd.tensor_sub(feo_q[m][:, :], pa[:, :], pb[:, :])

    # k-side: kT is HOST-folded (kTo|kTeo|kTee) — projection is linear, so
    # these chunks ARE the DIT folds of kh; no device fold ops for k.
    for m in range(TM):
        ps = ps1.tile([128, D], F32, tag="p1", name="p1")
        for kc in range(KC):
            nc.tensor.matmul(
                ps[:, :], kt[kc][:, ts(m, 128)], wk[kc],
                start=(kc == 0), stop=(kc == KC - 1),
            )
        nc.scalar.activation(ktgt[m][:, :], ps[:, :], AF.Copy)
    # qh_t[c, t] channel-major via PE transposes of the fp16 htd tiles
    # (1 cycle/row, 6x cheaper than re-projecting), DVE drains PSUM, then
    # doubled into q2 for the mod-L gathers.
    ps1t = tc.alloc_tile_pool(name="ps1t", bufs=2, space="PSUM")
    for mc in range(CN):
        qht = qht_pool.tile([128, L], F16, tag="qht", name="qht")
        for jg in range(4):
            pt = ps1t.tile([128, 512], F16, tag="pt", name="pt")
            for jj in range(4):
                m = 4 * jg + jj
                nc.tensor.transpose(
                    pt[:, ts(jj, 128)], htd_q[m][:, ts(mc, 128)], ident
                )
            nc.vector.tensor_scalar_add(
                qht[:, ts(jg, 512)], pt[:, :], bqcol[:, mc : mc + 1]
            )
        nc.scalar.dma_start(q2[ts(mc, 128), 0:L], qht[:, :])
        nc.scalar.dma_start(q2[ts(mc, 128), L : 2 * L], qht[:, :])

    ps1t.release()
    qht_pool.release()
    ps1.release()
    wqk_pool.release()
    qt_pool.release()

    # ---- S3+S4 fused: forward DFT with inline freq product ----
    # Qhat[fs, c] = sum_t Cf[t, fs] * qh_td[t, c]; pairs (j, 9+j) are produced
    # back-to-back so Z = Qhat * conj(Khat) is computed inline and the big
    # Qhat/Khat buffers never materialize.
    s_pool0 = tc.alloc_tile_pool(name="small0", bufs=1)
    iobs = []
    for mc in range(CN):
        iob = s_pool0.tile([128, 8], U32, tag=f"io{mc}", name=f"io{mc}")
        nc.gpsimd.iota(
            iob[:, :], pattern=[[0, 8]], base=mc * 128 * 2 * L,
            channel_multiplier=2 * L,
        )
        iobs.append(iob)
    # resident inverse branch bases (3 MB total, Act HWDGE queue)
    mi_pool = tc.alloc_tile_pool(name="mi", bufs=1)
    ib1 = [mi_pool.tile([128, 1024], F16, tag=f"i1{n}", name=f"i1{n}") for n in range(8)]
    ib2 = [mi_pool.tile([128, 512], F16, tag=f"i2{n}", name=f"i2{n}") for n in range(4)]
    ib3 = [mi_pool.tile([128, 512], F16, tag=f"i3{n}", name=f"i3{n}") for n in range(4)]
    for n in range(8):
        nc.scalar.dma_start(ib1[n][:, :], dr["IB1"][ts(n, 128), :])
    for n in range(4):
        nc.scalar.dma_start(ib2[n][:, :], dr["IB2"][ts(n, 128), :])
        nc.scalar.dma_start(ib3[n][:, :], dr["IB3"][ts(n, 128), :])

    z_pool = tc.alloc_tile_pool(name="zfreq", bufs=1)
    f_pool = tc.alloc_tile_pool(name="fpair", bufs=2)
    ps3 = tc.alloc_tile_pool(name="ps3", bufs=3, space="PSUM")

    Z = [z_pool.tile([128, D], F16, tag=f"z{j}", name=f"z{j}") for j in range(FM)]

    def proj_chunk(dname, oc, n_kc, fq, fk):
        psq = ps3.tile([128, D], F32, tag="p3q", name="p3q")
        psk = ps3.tile([128, D], F32, tag="p3k", name="p3k")
        bt = cf_pool.tile([128, n_kc * 128], F16, tag=f"cf{n_kc}", name=f"cf{n_kc}")
        nc.sync.dma_start(bt[:, :], dr[dname][ts(oc, 128), :])
        for kc in range(n_kc):
            nc.tensor.matmul(
                psq[:, :], bt[:, ts(kc, 128)], fq[kc][:, :],
                start=(kc == 0), stop=(kc == n_kc - 1),
            )
            nc.tensor.matmul(
                psk[:, :], bt[:, ts(kc, 128)], fk[kc][:, :],
                start=(kc == 0), stop=(kc == n_kc - 1),
            )
        return psq, psk

    # (basis, Z-chunk re, Z-chunk im, basis oc re, oc im, n_kc, folds, special)
    PAIRS = (
        [("B1", j, 4 + j, j, 4 + j, 8, fo_q, fo_k, False) for j in range(4)]
        + [("B2", 8 + j, 10 + j, j, 2 + j, 4, feo_q, feo_k, False) for j in range(2)]
        + [("B3", 12 + j, 14 + j, j, 2 + j, 4, fee_q, fee_k, j == 0) for j in range(2)]
    )
    for dname, re, im, ocr, oci, n_kc, fq, fk, special in PAIRS:
        psq_a, psk_a = proj_chunk(dname, ocr, n_kc, fq, fk)
        psq_b, psk_b = proj_chunk(dname, oci, n_kc, fq, fk)
        # wide-product formulation: qcomb = (Qre|Qim), qswap = (Qim|Qre),
        # kcomb = (Kre|Kim) * 2/L. Then P1 = qcomb*kcomb gives (QreKre|QimKim)
        # and P2 = qswap*kcomb gives (QimKre|QreKim):
        #   Zre = P1a + P1b, Znim = P2a - P2b — 4 DVE ops instead of 6.
        qcomb = f_pool.tile([128, 2 * D], F16, tag="qc", name="qc")
        qswap = f_pool.tile([128, 2 * D], F16, tag="qs", name="qs")
        kcomb = f_pool.tile([128, 2 * D], F16, tag="kc2", name="kc2")
        nc.scalar.activation(qcomb[:, 0:D], psq_a[:, :], AF.Copy)
        nc.scalar.activation(qswap[:, D : 2 * D], psq_a[:, :], AF.Copy)
        nc.scalar.activation(qcomb[:, D : 2 * D], psq_b[:, :], AF.Copy)
        nc.scalar.activation(qswap[:, 0:D], psq_b[:, :], AF.Copy)
        nc.scalar.activation(kcomb[:, 0:D], psk_a[:, :], AF.Copy, scale=2.0 / L)
        nc.scalar.activation(kcomb[:, D : 2 * D], psk_b[:, :], AF.Copy, scale=2.0 / L)
        if special:
            nc.vector.tensor_add(qcomb[0:1, 0:D], qcomb[0:1, 0:D], brow["bqL"][:, :])
            nc.vector.tensor_add(
                qswap[0:1, D : 2 * D], qswap[0:1, D : 2 * D], brow["bqL"][:, :]
            )
            nc.vector.tensor_add(kcomb[0:1, 0:D], kcomb[0:1, 0:D], brow["bkL2"][:, :])
        P1 = f_pool.tile([128, 2 * D], F16, tag="pp", name="pp")
        P2 = f_pool.tile([128, 2 * D], F16, tag="pp", name="pp")
        nc.vector.tensor_mul(P1[:, :], qcomb[:, :], kcomb[:, :])
        nc.gpsimd.tensor_mul(P2[:, :], qswap[:, :], kcomb[:, :])
        nc.vector.tensor_add(Z[re][:, :], P1[:, 0:D], P1[:, D : 2 * D])
        nc.gpsimd.tensor_sub(Z[im][:, :], P2[:, 0:D], P2[:, D : 2 * D])
        if special:
            # row 0: DC = Qre0*Kre0 (= P1 left half) and Nyquist = Qim0*Kim0
            # (= P1 right half); both 1/L-scaled, kcomb carries 2/L -> halve.
            nc.vector.tensor_scalar_mul(Z[re][0:1, :], P1[0:1, 0:D], 0.5)
            nc.vector.tensor_scalar_mul(Z[im][0:1, :], P1[0:1, D : 2 * D], 0.5)

    ps3.release()
    f_pool.release()
    htd_pool.release()
    cf_pool.release()

    # ---- S5/S6/S7 interleaved per channel chunk ----
    # inv-DFT(mc) on the PE; then its top-k + gather launches (DVE + SWDGE)
    # overlap inv-DFT(mc+1); wsum(mc) fills the PSUM-copy window of
    # inv-DFT(mc+2). Weights are folded into diag(w) fp16 stationaries.
    r_pool = tc.alloc_tile_pool(name="rcorr", bufs=1, side="right")
    psa = tc.alloc_tile_pool(name="psa", bufs=4, space="PSUM")
    ps5 = tc.alloc_tile_pool(name="ps5", bufs=4, space="PSUM")
    s_pool = tc.alloc_tile_pool(name="small", bufs=1)
    acc_pool = tc.alloc_tile_pool(name="acc", bufs=1, side="right")
    g_pool = tc.alloc_tile_pool(name="g", bufs=6)
    dg_pool = tc.alloc_tile_pool(name="dg", bufs=12)

    R = [r_pool.tile([128, L], F32, tag=f"r{m}", name=f"r{m}") for m in range(CN)]
    cand = [s_pool0.tile([128, 32], F32, tag=f"c{m}", name=f"c{m}") for m in range(CN)]
    acc = [acc_pool.tile([128, L], F16, tag=f"a{mc}", name=f"a{mc}") for mc in range(CN)]

    u_pool = tc.alloc_tile_pool(name="u", bufs=8)
    ut_pool = tc.alloc_tile_pool(name="ut", bufs=4)

    def inv_dft(mc):
        # branch inverses (stage A, PE) ...
        p1a = ps5.tile([128, 512], F32, tag="p5", name="p5")
        p1b = ps5.tile([128, 512], F32, tag="p5", name="p5")
        for fc in range(8):
            nc.tensor.matmul(
                p1a[:, :], Z[fc][:, ts(mc, 128)], ib1[fc][:, 0:512],
                start=(fc == 0), stop=(fc == 7),
            )
            nc.tensor.matmul(
                p1b[:, :], Z[fc][:, ts(mc, 128)], ib1[fc][:, 512:1024],
                start=(fc == 0), stop=(fc == 7),
            )
        p2 = ps5.tile([128, 512], F32, tag="p5", name="p5")
        for i in range(4):
            nc.tensor.matmul(
                p2[:, :], Z[8 + i][:, ts(mc, 128)], ib2[i][:, :],
                start=(i == 0), stop=(i == 3),
            )
        p3 = ps5.tile([128, 512], F32, tag="p5", name="p5")
        for i in range(4):
            nc.tensor.matmul(
                p3[:, :], Z[12 + i][:, ts(mc, 128)], ib3[i][:, :],
                start=(i == 0), stop=(i == 3),
            )
        u1a = u_pool.tile([128, 512], F16, tag="u", name="u")
        u1b = u_pool.tile([128, 512], F16, tag="u", name="u")
        u2 = u_pool.tile([128, 512], F16, tag="u", name="u")
        u3 = u_pool.tile([128, 512], F16, tag="u", name="u")
        nc.scalar.activation(u1a[:, :], p1a[:, :], AF.Copy)
        nc.scalar.activation(u1b[:, :], p1b[:, :], AF.Copy)
        nc.scalar.activation(u2[:, :], p2[:, :], AF.Copy)
        nc.scalar.activation(u3[:, :], p3[:, :], AF.Copy)
        # ... then the 4-way unfold (stage B, DVE/gpsimd):
        # r[k*512:...] = (-1)^(k>=2) u1[k%2] + (-1)^k u2 + u3
        tp = ut_pool.tile([128, 512], F16, tag="ut", name="ut")
        tm_ = ut_pool.tile([128, 512], F16, tag="ut", name="ut")
        nc.vector.tensor_add(tp[:, :], u3[:, :], u2[:, :])
        nc.gpsimd.tensor_sub(tm_[:, :], u3[:, :], u2[:, :])
        nc.vector.tensor_add(R[mc][:, 0:512], tp[:, :], u1a[:, :])
        nc.gpsimd.tensor_add(R[mc][:, ts(1, 512)], tm_[:, :], u1b[:, :])
        nc.vector.tensor_sub(R[mc][:, ts(2, 512)], tp[:, :], u1a[:, :])
        nc.gpsimd.tensor_sub(R[mc][:, ts(3, 512)], tm_[:, :], u1b[:, :])
        for n in range(4):
            nc.vector.max(out=cand[mc][:, ts(n, 8)], in_=R[mc][:, ts(n, 512)])

    def topk_gather(mc):
        vals = s_pool.tile([128, 8], F32, tag=f"v{mc}", name=f"v{mc}")
        nc.vector.max(out=vals[:, :], in_=cand[mc][:, :])
        idx = s_pool.tile([128, 8], U32, tag=f"i{mc}", name=f"i{mc}")
        nc.vector.max_index(out=idx[:, :], in_max=vals[:, :], in_values=R[mc][:, :])
        off = s_pool.tile([128, 8], U32, tag=f"o{mc}", name=f"o{mc}")
        nc.vector.tensor_add(off[:, :], idx[:, :], iobs[mc][:, :])
        gs = []
        for k in range(TOPK):
            g = g_pool.tile([128, L], F16, tag="g", name="g")
            gi = nc.gpsimd.indirect_dma_start(
                out=g[:, :],
                out_offset=None,
                in_=q2[:, :],
                in_offset=IndirectOffsetOnAxis(ap=off[:, k : k + 1], axis=1),
            )
            if k % 4:
                gi.ins.queue = f"qPoolDynamic{k % 4}"
            gs.append(g)
        negm = s_pool.tile([128, 1], F32, tag=f"nm{mc}", name=f"nm{mc}")
        nc.vector.tensor_scalar_mul(negm[:, :], vals[:, 0:1], -1.0)
        e = s_pool.tile([128, 8], F32, tag=f"e{mc}", name=f"e{mc}")
        nc.scalar.activation(e[:, :], vals[:, :], AF.Exp, bias=negm[:, :])
        ssum = s_pool.tile([128, 1], F32, tag=f"s{mc}", name=f"s{mc}")
        nc.vector.reduce_sum(out=ssum[:, :], in_=e[:, :], axis=AX.X)
        rs = s_pool.tile([128, 1], F32, tag=f"rs{mc}", name=f"rs{mc}")
        nc.vector.reciprocal(rs[:, :], ssum[:, :])
        wt = s_pool.tile([128, 8], F32, tag=f"w{mc}", name=f"w{mc}")
        nc.vector.tensor_scalar_mul(wt[:, :], e[:, :], rs[:, :])
        ds = []
        for k in range(TOPK):
            dg = dg_pool.tile([128, 128], F16, tag="dg", name="dg")
            nc.vector.tensor_scalar_mul(dg[:, :], ident[:, :], wt[:, k : k + 1])
            ds.append(dg)
        return gs, ds

    def wsum(mc, gs, ds):
        pacc = [psa.tile([128, 512], F32, tag="pa", name="pa") for _ in range(4)]
        for k in range(TOPK):
            for nsl in range(4):
                nc.tensor.matmul(
                    pacc[nsl][:, :], ds[k][:, :], gs[k][:, ts(nsl, 512)],
                    start=(k == 0), stop=(k == TOPK - 1),
                )
        for nsl in range(4):
            nc.scalar.activation(acc[mc][:, ts(nsl, 512)], pacc[nsl][:, :], AF.Copy)

    gd = {}
    inv_dft(0)
    gd[0] = topk_gather(0)
    inv_dft(1)
    gd[1] = topk_gather(1)
    wsum(0, *gd[0])
    inv_dft(2)
    gd[2] = topk_gather(2)
    wsum(1, *gd[1])
    inv_dft(3)
    gd[3] = topk_gather(3)
    wsum(2, *gd[2])

    ps5.release()
    po_pool = tc.alloc_tile_pool(name="po", bufs=1, space="PSUM")
    ot_pool = tc.alloc_tile_pool(name="ot", bufs=4, side="right")

    wsum(3, *gd[3])

    # ---- S8: output projection, TRANSPOSED: outT[c, t] = sum_cin Wo[cin, c]
    # * acc[cin, t] + bo[c]. Channel-major output puts the bias on the
    # partition axis (fused into the PSUM copy); the host un-transposes.
    for cb in range(4):
        pss = [po_pool.tile([128, 512], F32, tag=f"po{tb}", name=f"po{tb}")
               for tb in range(4)]
        for kc in range(CN):
            for tb in range(4):
                nc.tensor.matmul(
                    pss[tb][:, :], wo[kc][:, ts(cb, 128)], acc[kc][:, ts(tb, 512)],
                    start=(kc == 0), stop=(kc == CN - 1),
                )
        for tb in range(4):
            ot = ot_pool.tile([128, 512], F16, tag="ot", name="ot")
            nc.scalar.activation(
                ot[:, :], pss[tb][:, :], AF.Identity, bias=bocol[:, cb : cb + 1]
            )
            eng = nc.sync if tb % 2 == 0 else nc.scalar
            eng.dma_start(out_ap[ts(cb, 128), ts(tb, 512)], ot[:, :])

    ot_pool.release()
    po_pool.release()
    psa.release()
    ut_pool.release()
    u_pool.release()
    dg_pool.release()
    g_pool.release()
    s_pool.release()
    z_pool.release()
    mi_pool.release()
    s_pool0.release()
    acc_pool.release()
    r_pool.release()
    w_pool.release()


def build_module():
    nc = bacc.Bacc(
        "TRN2",
        target_bir_lowering=False,
        debug=False,
        enable_asserts=False,
        num_devices=N_CORES,
        num_swdge_queues=4,
    )
    dr = {}

    def din(name, shape, dt=F32R):
        dr[name] = nc.dram_tensor(name, shape, dt, kind="ExternalInput").ap()

    din("qT", [D, L], F16)
    din("kT", [D, L], F16)
    din("Wq", [128, KC * D], F16)   # tiled: [p, kc*D+j] = W[kc*128+p, j]
    din("Wk", [128, KC * D], F16)
    din("Wo", [128, KC * D], F16)
    din("bqL", [1, D], F16)
    din("bkL2", [1, D], F16)
    din("bqc", [D, 1], F32)
    din("boc", [D, 1], F32)
    din("ident", [128, 128], F16)
    din("B1", [8 * 128, 8 * 128], F16)
    din("B2", [4 * 128, 4 * 128], F16)
    din("B3", [4 * 128, 4 * 128], F16)
    din("IB1", [8 * 128, 1024], F16)
    din("IB2", [4 * 128, 512], F16)
    din("IB3", [4 * 128, 512], F16)
    out_ap = nc.dram_tensor("out", [D, L], F16, kind="ExternalOutput").ap()
    q2 = nc.dram_tensor("q2", [D, 2 * L], F16, kind="Internal").ap()

    with tile.TileContext(nc, trace_sim=False) as tc:
        _kernel_body(tc, dr, out_ap, q2)
    nc.compile()
    return nc


_NC_CACHE = {}


def _tile_w(W):
    return np.ascontiguousarray(
        np.asarray(W, np.float32).reshape(KC, 128, D).transpose(1, 0, 2).reshape(128, KC * D)
    )


def make_in_maps(q, k, Wq, bq, Wk, bk, Wo, bo):
    B1, B2, B3 = _build_dft_mats()

    def tile_fwd(Bm, nch):
        return np.ascontiguousarray(
            Bm.reshape(nch, 128, nch, 128).transpose(2, 1, 0, 3)
            .reshape(nch * 128, nch * 128)
        ).astype(np.float16)

    f32 = np.float32
    shared = {
        "Wq": _tile_w(Wq).astype(np.float16),
        "Wk": _tile_w(Wk).astype(np.float16),
        "Wo": _tile_w(Wo).astype(np.float16),
        "bqL": (np.asarray(bq, f32) * L).reshape(1, D).astype(np.float16),
        "bkL2": (np.asarray(bk, f32) * 2.0).reshape(1, D).astype(np.float16),
        "bqc": np.ascontiguousarray(bq, f32).reshape(D, 1),
        "boc": np.ascontiguousarray(bo, f32).reshape(D, 1),
        "ident": np.eye(128, dtype=np.float16),
        "B1": tile_fwd(B1, 8),
        "B2": tile_fwd(B2, 4),
        "B3": tile_fwd(B3, 4),
        "IB1": np.ascontiguousarray(B1.T).astype(np.float16),
        "IB2": np.ascontiguousarray(B2.T).astype(np.float16),
        "IB3": np.ascontiguousarray(B3.T).astype(np.float16),
    }
    in_maps = []
    for b in range(B):
        m = dict(shared)
        m["qT"] = np.ascontiguousarray(np.asarray(q[b], f32).T).astype(np.float16)
        kt_ = np.asarray(k[b], f32).T
        ko = kt_[:, :1024] - kt_[:, 1024:]
        ke = kt_[:, :1024] + kt_[:, 1024:]
        m["kT"] = np.ascontiguousarray(np.concatenate(
            [ko, ke[:, :512] - ke[:, 512:], ke[:, :512] + ke[:, 512:]], axis=1
        )).astype(np.float16)
        in_maps.append(m)
    return in_maps


def kernel(q, k, v, Wq, bq, Wk, bk, Wv, bv, Wo, bo, _want_results=False,
           _trace=False, **_ignored):
    if "nc" not in _NC_CACHE:
        _NC_CACHE["nc"] = build_module()
    nc = _NC_CACHE["nc"]
    in_maps = make_in_maps(q, k, Wq, bq, Wk, bk, Wo, bo)
    # warmup execution: the first run of a freshly-loaded NEFF on a core that
    # ran a different program can read stale state; run once and discard.
    run_bass_kernel_spmd(nc, in_maps, core_ids=list(range(N_CORES)), trace=False)
    res = run_bass_kernel_spmd(
        nc, in_maps, core_ids=list(range(N_CORES)), trace=_trace
    )
    out = np.stack([np.asarray(res.results[b]["out"], np.float32).T for b in range(B)])
    out = np.ascontiguousarray(out)
    if _want_results:
        return out, res
    return out


if __name__ == "__main__":
    # smoke test with random data
    rng = np.random.default_rng(0)
    q = rng.standard_normal((B, L, D), np.float32)
    k = rng.standard_normal((B, L, D), np.float32)
    s = 1.0 / np.sqrt(D)
    Wq = rng.standard_normal((D, D), np.float32) * s
    Wk = rng.standard_normal((D, D), np.float32) * s
    Wo = rng.standard_normal((D, D), np.float32) * s
    z = np.zeros(D, np.float32)
    out = kernel(q, k, None, Wq, z, Wk, z, None, None, Wo, z)
    print("out", out.shape, out.dtype, float(np.abs(out).sum()))



# revision 52
# speedup vs baseline: 1.0105x; 1.0105x over previous
"""AutoCorrelation block (FFT cross-correlation attention) on 8 Trainium2 cores.

Math (per batch b, faithfully reproducing the reference):
  qh = q @ Wq + bq, kh = k @ Wk + bk         (v projection is dead code)
  per channel c=(h,dh) (512 per batch):
    r = irfft(rfft(qh_c) * conj(rfft(kh_c)))   # circular cross-correlation
    top-8 lags d_k of r, softmax of the 8 values -> w_k
    agg_c[t] = sum_k w_k * qh_c[(t + d_k) % L]
  out = agg^T @ Wo + bo

Implementation: DFT-as-matmul with a stacked real cos/sin basis (shared by all
channels, so the whole FFT pipeline is dense PE work). All matmul operands are
fp16 (fp32 PSUM accumulation): fp16 moving operands run at 1 cycle/row and the
quantization error (~5e-3 on the final output) is well inside the 2e-2 gate.
Key structure:
  - projections with quarter-tile-paced input DMA (PE starts after ~0.8 MB);
  - channel-major qh (gather source) via PE transposes of the fp16 htd tiles
    with the bias fused into the DVE PSUM-drain (per-partition add);
  - forward DFT with the frequency product computed inline (Z never leaves
    SBUF, fp16, 2/L irfft scale folded into the kf copy so the inverse basis
    is unscaled cos/sin — exact in fp16 and fully SBUF-resident at 8 MB);
  - inverse DFT interleaved per 128-channel chunk with top-8 (DVE max/
    max_index), softmax, diag(w) stationary construction, indirect-DMA
    gathers (4 SWDGE queues) and the weighted sum as PE matmuls with
    diag(w_k) stationaries — gathers of chunk i overlap inv-DFT of chunk i+1;
  - output projection computed TRANSPOSED (channel-major) so bo is a
    per-partition bias fused into the PSUM copy; the host un-transposes.

Sharding: data-parallel over batch. B == 8 == n_cores, one batch per core,
weights + DFT matrices replicated. No collectives.
"""

import numpy as np

import concourse.bass as bass
import concourse.bacc as bacc
import concourse.mybir as mybir
import concourse.tile as tile
from concourse.bass import IndirectOffsetOnAxis, ts
from concourse.bass_utils import run_bass_kernel_spmd

B, L, D = 8, 2048, 512
TOPK = 8
NF = 1025          # rfft bins for L=2048
FS = 2048          # stacked freq rows: 16 chunks of 128
IM0 = 1024         # sin(f) block at 1024+f (f=1..1023); slot 1024 = Nyquist cos
N_CORES = 8
KC = 4             # d_in chunks of 128
TM = 16            # time chunks of 128
CN = 4             # channel chunks of 128
FM = 16            # stacked-freq chunks of 128

F32 = mybir.dt.float32
F32R = mybir.dt.float32r
U32 = mybir.dt.uint32
BF16 = mybir.dt.bfloat16
F16 = mybir.dt.float16
AF = mybir.ActivationFunctionType
AX = mybir.AxisListType


def _build_dft_mats():
    # Two-level DIT split: all folds are partition-aligned tile adds, all
    # twiddles absorbed into three branch-specific real bases.
    #   b1: odd bins f=2i+1       over xo  = x[:1024] - x[1024:]      (8 chunks)
    #   b2: f=2h, h odd           over xeo = fold2(xe) minus          (4 chunks)
    #   b3: f=4g (incl DC/Nyq)    over xee = fold2(xe) plus           (4 chunks)
    # Chunk-pair stacking (re, im) mirrors the original full basis; b3 keeps
    # the DC row and stores the Nyquist cos row in the sin(0) slot.
    t1 = np.arange(1024)
    t2 = np.arange(512)
    h1 = 2 * np.arange(512) + 1
    B1 = np.zeros((1024, 1024), np.float32)
    a1 = (2.0 * np.pi / 2048.0) * (np.outer(t1, h1) % 2048)
    B1[:, :512] = np.cos(a1)
    B1[:, 512:] = np.sin(a1)
    h2 = 2 * np.arange(256) + 1
    B2 = np.zeros((512, 512), np.float32)
    a2 = (2.0 * np.pi / 1024.0) * (np.outer(t2, h2) % 1024)
    B2[:, :256] = np.cos(a2)
    B2[:, 256:] = np.sin(a2)
    g3 = np.arange(256)
    B3 = np.zeros((512, 512), np.float32)
    a3 = (2.0 * np.pi / 512.0) * (np.outer(t2, g3) % 512)
    B3[:, :256] = np.cos(a3)
    B3[:, 256] = np.where(t2 % 2 == 0, 1.0, -1.0)
    B3[:, 257:] = np.sin(a3[:, 1:])
    return B1, B2, B3


def _kernel_body(tc, dr, out_ap, q2):
    nc = tc.nc

    w_pool = tc.alloc_tile_pool(name="weights", bufs=1)
    cf_pool = tc.alloc_tile_pool(name="cf", bufs=4, side="right")
    htd_pool = tc.alloc_tile_pool(name="htd", bufs=1, side="right")

    # ---- S1 inputs first so the PE can start ASAP ----
    qt_pool = tc.alloc_tile_pool(name="qt", bufs=1)
    qt = [qt_pool.tile([128, L], F16, tag=f"qt{i}", name=f"qt{i}") for i in range(KC)]
    kt = [qt_pool.tile([128, L], F16, tag=f"kt{i}", name=f"kt{i}") for i in range(KC)]

    # ---- constants (DMA order matters: the sync queue is in-order, so load
    # exactly what the first matmul group needs first) ----
    wqk_pool = tc.alloc_tile_pool(name="wqk", bufs=1)
    wq_t = wqk_pool.tile([128, KC * D], F16, tag="wqt", name="wqt")
    wk_t = wqk_pool.tile([128, KC * D], F16, tag="wkt", name="wkt")
    wo_t = w_pool.tile([128, KC * D], F16, tag="wot", name="wot")
    # tiny constants first (37 KB): bias rows and ident must not sit behind
    # megabyte loads — the grp-0 bias matmul needs them at ~14us.
    ident = w_pool.tile([128, 128], F16, tag="ident", name="ident")
    nc.sync.dma_start(ident[:, :], dr["ident"][:, :])
    brow = {}
    for nm in ("bqL", "bkL2"):
        brow[nm] = w_pool.tile([1, D], F16, tag=f"{nm}r", name=f"{nm}r")
        nc.sync.dma_start(brow[nm][:, :], dr[nm][:, :])
    bqcol = w_pool.tile([128, CN], F32, tag="bqc", name="bqc")
    for c in range(CN):
        nc.sync.dma_start(bqcol[:, c : c + 1], dr["bqc"][ts(c, 128), :])
    bocol = w_pool.tile([128, CN], F32, tag="boc", name="boc")
    for c in range(CN):
        nc.sync.dma_start(bocol[:, c : c + 1], dr["boc"][ts(c, 128), :])
    # quarter-tile interleaved loads: the first matmul group needs ~0.8 MB
    for i in range(KC):
        nc.sync.dma_start(qt[i][:, 0:512], dr["qT"][ts(i, 128), 0:512])
        nc.sync.dma_start(wq_t[:, ts(i, D)], dr["Wq"][:, ts(i, D)])
    for q4 in range(1, 4):
        for i in range(KC):
            nc.sync.dma_start(qt[i][:, ts(q4, 512)], dr["qT"][ts(i, 128), ts(q4, 512)])
    # k-side after q-side on the same queue: full bandwidth for the critical
    # path, and kt still lands well before the kh projection (~31us)
    for i in range(KC):
        nc.sync.dma_start(wk_t[:, ts(i, D)], dr["Wk"][:, ts(i, D)])
        nc.sync.dma_start(kt[i][:, 0:512], dr["kT"][ts(i, 128), 0:512])
    for q4 in range(1, 4):
        for i in range(KC):
            nc.sync.dma_start(kt[i][:, ts(q4, 512)], dr["kT"][ts(i, 128), ts(q4, 512)])
    nc.scalar.dma_start(wo_t[:, :], dr["Wo"][:, :])
    wq = [wq_t[:, ts(i, D)] for i in range(KC)]
    wk = [wk_t[:, ts(i, D)] for i in range(KC)]
    wo = [wo_t[:, ts(i, D)] for i in range(KC)]

    htd_q = [htd_pool.tile([128, D], F16, tag=f"hq{m}", name=f"hq{m}") for m in range(TM)]
    fo_k = [htd_pool.tile([128, D], F16, tag=f"fok{m}", name=f"fok{m}") for m in range(8)]
    feo_k = [htd_pool.tile([128, D], F16, tag=f"eok{m}", name=f"eok{m}") for m in range(4)]
    fee_k = [htd_pool.tile([128, D], F16, tag=f"eek{m}", name=f"eek{m}") for m in range(4)]
    ktgt = fo_k + feo_k + fee_k

    # ---- S1/S2: projections (all-fp16 operands, fp32 PSUM accumulate) ----
    ps1 = tc.alloc_tile_pool(name="ps1", bufs=6, space="PSUM")
    qht_pool = tc.alloc_tile_pool(name="qht", bufs=2)

    # qh_td[t, c] = sum_di qT[di, t] * Wq[di, c]; bias is applied in the
    # channel-major transpose copies (per-partition there) and via the DC-bin
    # fix in the forward DFT.
    for grp in range(4):
        pss1 = [ps1.tile([128, D], F32, tag="p1", name="p1") for _ in range(4)]
        for kc in range(KC):
            for m4 in range(4):
                nc.tensor.matmul(
                    pss1[m4][:, :], qt[kc][:, ts(grp * 4 + m4, 128)], wq[kc],
                    start=(kc == 0), stop=(kc == KC - 1),
                )
        for m4 in range(4):
            nc.scalar.activation(
                htd_q[grp * 4 + m4][:, :], pss1[m4][:, :], AF.Copy
            )
    # k-side: kT is HOST-folded (kTo|kTeo|kTee) — projection is linear, so
    # these chunks ARE the DIT folds of kh; no device fold ops for k.
    for m in range(TM):
        ps = ps1.tile([128, D], F32, tag="p1", name="p1")
        for kc in range(KC):
            nc.tensor.matmul(
                ps[:, :], kt[kc][:, ts(m, 128)], wk[kc],
                start=(kc == 0), stop=(kc == KC - 1),
            )
        nc.scalar.activation(ktgt[m][:, :], ps[:, :], AF.Copy)
    # qh_t[c, t] channel-major via PE transposes of the fp16 htd tiles
    # (1 cycle/row, 6x cheaper than re-projecting), DVE drains PSUM, then
    # doubled into q2 for the mod-L gathers.
    ps1t = tc.alloc_tile_pool(name="ps1t", bufs=2, space="PSUM")
    for mc in range(CN):
        qht = qht_pool.tile([128, L], F16, tag="qht", name="qht")
        for jg in range(4):
            pt = ps1t.tile([128, 512], F16, tag="pt", name="pt")
            for jj in range(4):
                m = 4 * jg + jj
                nc.tensor.transpose(
                    pt[:, ts(jj, 128)], htd_q[m][:, ts(mc, 128)], ident
                )
            nc.vector.tensor_scalar_add(
                qht[:, ts(jg, 512)], pt[:, :], bqcol[:, mc : mc + 1]
            )
        nc.scalar.dma_start(q2[ts(mc, 128), 0:L], qht[:, :])
        nc.scalar.dma_start(q2[ts(mc, 128), L : 2 * L], qht[:, :])

    ps1t.release()
    qht_pool.release()
    ps1.release()
    wqk_pool.release()
    qt_pool.release()

    # ---- S3+S4 fused: forward DFT with inline freq product ----
    # Qhat[fs, c] = sum_t Cf[t, fs] * qh_td[t, c]; pairs (j, 9+j) are produced
    # back-to-back so Z = Qhat * conj(Khat) is computed inline and the big
    # Qhat/Khat buffers never materialize.
    s_pool0 = tc.alloc_tile_pool(name="small0", bufs=1)
    iobs = []
    for mc in range(CN):
        iob = s_pool0.tile([128, 8], U32, tag=f"io{mc}", name=f"io{mc}")
        nc.gpsimd.iota(
            iob[:, :], pattern=[[0, 8]], base=mc * 128 * 2 * L,
            channel_multiplier=2 * L,
        )
        iobs.append(iob)
    # resident inverse branch bases (3 MB total, Act HWDGE queue)
    mi_pool = tc.alloc_tile_pool(name="mi", bufs=1)
    ib1 = [mi_pool.tile([128, 1024], F16, tag=f"i1{n}", name=f"i1{n}") for n in range(8)]
    ib2 = [mi_pool.tile([128, 512], F16, tag=f"i2{n}", name=f"i2{n}") for n in range(4)]
    ib3 = [mi_pool.tile([128, 512], F16, tag=f"i3{n}", name=f"i3{n}") for n in range(4)]
    for n in range(8):
        nc.scalar.dma_start(ib1[n][:, :], dr["IB1"][ts(n, 128), :])
    for n in range(4):
        nc.scalar.dma_start(ib2[n][:, :], dr["IB2"][ts(n, 128), :])
        nc.scalar.dma_start(ib3[n][:, :], dr["IB3"][ts(n, 128), :])

    # ---- DIT folds: partition-aligned tile adds (t and t+1024 share the
    # partition), spread over DVE and gpsimd. fo feeds the odd-bin branch,
    # fee/feo the two level-2 branches.
    fold_pool = tc.alloc_tile_pool(name="fold", bufs=1)
    ftmp_pool = tc.alloc_tile_pool(name="ftmp", bufs=3)
    fo_q = [fold_pool.tile([128, D], F16, tag=f"foq{m}", name=f"foq{m}") for m in range(8)]
    fee_q = [fold_pool.tile([128, D], F16, tag=f"eeq{m}", name=f"eeq{m}") for m in range(4)]
    feo_q = [fold_pool.tile([128, D], F16, tag=f"eoq{m}", name=f"eoq{m}") for m in range(4)]
    for m in range(8):
        eng = nc.vector if m % 2 == 0 else nc.gpsimd
        eng.tensor_sub(fo_q[m][:, :], htd_q[m][:, :], htd_q[m + 8][:, :])
    for m in range(4):
        pa = ftmp_pool.tile([128, D], F16, tag="pa", name="pa")
        pb = ftmp_pool.tile([128, D], F16, tag="pb", name="pb")
        nc.vector.tensor_add(pa[:, :], htd_q[m][:, :], htd_q[m + 8][:, :])
        nc.gpsimd.tensor_add(pb[:, :], htd_q[m + 4][:, :], htd_q[m + 12][:, :])
        nc.vector.tensor_add(fee_q[m][:, :], pa[:, :], pb[:, :])
        nc.gpsimd.tensor_sub(feo_q[m][:, :], pa[:, :], pb[:, :])

    z_pool = tc.alloc_tile_pool(name="zfreq", bufs=1)
    f_pool = tc.alloc_tile_pool(name="fpair", bufs=2)
    ps3 = tc.alloc_tile_pool(name="ps3", bufs=3, space="PSUM")

    Z = [z_pool.tile([128, D], F16, tag=f"z{j}", name=f"z{j}") for j in range(FM)]

    def proj_chunk(dname, oc, n_kc, fq, fk):
        psq = ps3.tile([128, D], F32, tag="p3q", name="p3q")
        psk = ps3.tile([128, D], F32, tag="p3k", name="p3k")
        bt = cf_pool.tile([128, n_kc * 128], F16, tag=f"cf{n_kc}", name=f"cf{n_kc}")
        nc.sync.dma_start(bt[:, :], dr[dname][ts(oc, 128), :])
        for kc in range(n_kc):
            nc.tensor.matmul(
                psq[:, :], bt[:, ts(kc, 128)], fq[kc][:, :],
                start=(kc == 0), stop=(kc == n_kc - 1),
            )
            nc.tensor.matmul(
                psk[:, :], bt[:, ts(kc, 128)], fk[kc][:, :],
                start=(kc == 0), stop=(kc == n_kc - 1),
            )
        return psq, psk

    # (basis, Z-chunk re, Z-chunk im, basis oc re, oc im, n_kc, folds, special)
    PAIRS = (
        [("B1", j, 4 + j, j, 4 + j, 8, fo_q, fo_k, False) for j in range(4)]
        + [("B2", 8 + j, 10 + j, j, 2 + j, 4, feo_q, feo_k, False) for j in range(2)]
        + [("B3", 12 + j, 14 + j, j, 2 + j, 4, fee_q, fee_k, j == 0) for j in range(2)]
    )
    for dname, re, im, ocr, oci, n_kc, fq, fk, special in PAIRS:
        psq_a, psk_a = proj_chunk(dname, ocr, n_kc, fq, fk)
        psq_b, psk_b = proj_chunk(dname, oci, n_kc, fq, fk)
        # wide-product formulation: qcomb = (Qre|Qim), qswap = (Qim|Qre),
        # kcomb = (Kre|Kim) * 2/L. Then P1 = qcomb*kcomb gives (QreKre|QimKim)
        # and P2 = qswap*kcomb gives (QimKre|QreKim):
        #   Zre = P1a + P1b, Znim = P2a - P2b — 4 DVE ops instead of 6.
        qcomb = f_pool.tile([128, 2 * D], F16, tag="qc", name="qc")
        qswap = f_pool.tile([128, 2 * D], F16, tag="qs", name="qs")
        kcomb = f_pool.tile([128, 2 * D], F16, tag="kc2", name="kc2")
        nc.scalar.activation(qcomb[:, 0:D], psq_a[:, :], AF.Copy)
        nc.scalar.activation(qswap[:, D : 2 * D], psq_a[:, :], AF.Copy)
        nc.scalar.activation(qcomb[:, D : 2 * D], psq_b[:, :], AF.Copy)
        nc.scalar.activation(qswap[:, 0:D], psq_b[:, :], AF.Copy)
        nc.scalar.activation(kcomb[:, 0:D], psk_a[:, :], AF.Copy, scale=2.0 / L)
        nc.scalar.activation(kcomb[:, D : 2 * D], psk_b[:, :], AF.Copy, scale=2.0 / L)
        if special:
            nc.vector.tensor_add(qcomb[0:1, 0:D], qcomb[0:1, 0:D], brow["bqL"][:, :])
            nc.vector.tensor_add(
                qswap[0:1, D : 2 * D], qswap[0:1, D : 2 * D], brow["bqL"][:, :]
            )
            nc.vector.tensor_add(kcomb[0:1, 0:D], kcomb[0:1, 0:D], brow["bkL2"][:, :])
        P1 = f_pool.tile([128, 2 * D], F16, tag="pp", name="pp")
        P2 = f_pool.tile([128, 2 * D], F16, tag="pp", name="pp")
        nc.vector.tensor_mul(P1[:, :], qcomb[:, :], kcomb[:, :])
        nc.gpsimd.tensor_mul(P2[:, :], qswap[:, :], kcomb[:, :])
        nc.vector.tensor_add(Z[re][:, :], P1[:, 0:D], P1[:, D : 2 * D])
        nc.gpsimd.tensor_sub(Z[im][:, :], P2[:, 0:D], P2[:, D : 2 * D])
        if special:
            # row 0: DC = Qre0*Kre0 (= P1 left half) and Nyquist = Qim0*Kim0
            # (= P1 right half); both 1/L-scaled, kcomb carries 2/L -> halve.
            nc.vector.tensor_scalar_mul(Z[re][0:1, :], P1[0:1, 0:D], 0.5)
            nc.vector.tensor_scalar_mul(Z[im][0:1, :], P1[0:1, D : 2 * D], 0.5)

    ps3.release()
    f_pool.release()
    htd_pool.release()
    cf_pool.release()

    # ---- S5/S6/S7 interleaved per channel chunk ----
    # inv-DFT(mc) on the PE; then its top-k + gather launches (DVE + SWDGE)
    # overlap inv-DFT(mc+1); wsum(mc) fills the PSUM-copy window of
    # inv-DFT(mc+2). Weights are folded into diag(w) fp16 stationaries.
    r_pool = tc.alloc_tile_pool(name="rcorr", bufs=1, side="right")
    psa = tc.alloc_tile_pool(name="psa", bufs=4, space="PSUM")
    ps5 = tc.alloc_tile_pool(name="ps5", bufs=4, space="PSUM")
    s_pool = tc.alloc_tile_pool(name="small", bufs=1)
    acc_pool = tc.alloc_tile_pool(name="acc", bufs=1, side="right")
    g_pool = tc.alloc_tile_pool(name="g", bufs=6)
    dg_pool = tc.alloc_tile_pool(name="dg", bufs=12)

    R = [r_pool.tile([128, L], F32, tag=f"r{m}", name=f"r{m}") for m in range(CN)]
    cand = [s_pool0.tile([128, 32], F32, tag=f"c{m}", name=f"c{m}") for m in range(CN)]
    acc = [acc_pool.tile([128, L], F16, tag=f"a{mc}", name=f"a{mc}") for mc in range(CN)]

    u_pool = tc.alloc_tile_pool(name="u", bufs=8)
    ut_pool = tc.alloc_tile_pool(name="ut", bufs=4)

    def inv_dft(mc):
        # branch inverses (stage A, PE) ...
        p1a = ps5.tile([128, 512], F32, tag="p5", name="p5")
        p1b = ps5.tile([128, 512], F32, tag="p5", name="p5")
        for fc in range(8):
            nc.tensor.matmul(
                p1a[:, :], Z[fc][:, ts(mc, 128)], ib1[fc][:, 0:512],
                start=(fc == 0), stop=(fc == 7),
            )
            nc.tensor.matmul(
                p1b[:, :], Z[fc][:, ts(mc, 128)], ib1[fc][:, 512:1024],
                start=(fc == 0), stop=(fc == 7),
            )
        p2 = ps5.tile([128, 512], F32, tag="p5", name="p5")
        for i in range(4):
            nc.tensor.matmul(
                p2[:, :], Z[8 + i][:, ts(mc, 128)], ib2[i][:, :],
                start=(i == 0), stop=(i == 3),
            )
        p3 = ps5.tile([128, 512], F32, tag="p5", name="p5")
        for i in range(4):
            nc.tensor.matmul(
                p3[:, :], Z[12 + i][:, ts(mc, 128)], ib3[i][:, :],
                start=(i == 0), stop=(i == 3),
            )
        u1a = u_pool.tile([128, 512], F16, tag="u", name="u")
        u1b = u_pool.tile([128, 512], F16, tag="u", name="u")
        u2 = u_pool.tile([128, 512], F16, tag="u", name="u")
        u3 = u_pool.tile([128, 512], F16, tag="u", name="u")
        nc.scalar.activation(u1a[:, :], p1a[:, :], AF.Copy)
        nc.scalar.activation(u1b[:, :], p1b[:, :], AF.Copy)
        nc.scalar.activation(u2[:, :], p2[:, :], AF.Copy)
        nc.scalar.activation(u3[:, :], p3[:, :], AF.Copy)
        # ... then the 4-way unfold (stage B, DVE/gpsimd):
        # r[k*512:...] = (-1)^(k>=2) u1[k%2] + (-1)^k u2 + u3
        tp = ut_pool.tile([128, 512], F16, tag="ut", name="ut")
        tm_ = ut_pool.tile([128, 512], F16, tag="ut", name="ut")
        nc.vector.tensor_add(tp[:, :], u3[:, :], u2[:, :])
        nc.gpsimd.tensor_sub(tm_[:, :], u3[:, :], u2[:, :])
        nc.vector.tensor_add(R[mc][:, 0:512], tp[:, :], u1a[:, :])
        nc.gpsimd.tensor_add(R[mc][:, ts(1, 512)], tm_[:, :], u1b[:, :])
        nc.vector.tensor_sub(R[mc][:, ts(2, 512)], tp[:, :], u1a[:, :])
        nc.gpsimd.tensor_sub(R[mc][:, ts(3, 512)], tm_[:, :], u1b[:, :])
        for n in range(4):
            nc.vector.max(out=cand[mc][:, ts(n, 8)], in_=R[mc][:, ts(n, 512)])

    def topk_gather(mc):
        vals = s_pool.tile([128, 8], F32, tag=f"v{mc}", name=f"v{mc}")
        nc.vector.max(out=vals[:, :], in_=cand[mc][:, :])
        idx = s_pool.tile([128, 8], U32, tag=f"i{mc}", name=f"i{mc}")
        nc.vector.max_index(out=idx[:, :], in_max=vals[:, :], in_values=R[mc][:, :])
        off = s_pool.tile([128, 8], U32, tag=f"o{mc}", name=f"o{mc}")
        nc.vector.tensor_add(off[:, :], idx[:, :], iobs[mc][:, :])
        gs = []
        for k in range(TOPK):
            g = g_pool.tile([128, L], F16, tag="g", name="g")
            gi = nc.gpsimd.indirect_dma_start(
                out=g[:, :],
                out_offset=None,
                in_=q2[:, :],
                in_offset=IndirectOffsetOnAxis(ap=off[:, k : k + 1], axis=1),
            )
            if k % 4:
                gi.ins.queue = f"qPoolDynamic{k % 4}"
            gs.append(g)
        negm = s_pool.tile([128, 1], F32, tag=f"nm{mc}", name=f"nm{mc}")
        nc.vector.tensor_scalar_mul(negm[:, :], vals[:, 0:1], -1.0)
        e = s_pool.tile([128, 8], F32, tag=f"e{mc}", name=f"e{mc}")
        nc.scalar.activation(e[:, :], vals[:, :], AF.Exp, bias=negm[:, :])
        ssum = s_pool.tile([128, 1], F32, tag=f"s{mc}", name=f"s{mc}")
        nc.vector.reduce_sum(out=ssum[:, :], in_=e[:, :], axis=AX.X)
        rs = s_pool.tile([128, 1], F32, tag=f"rs{mc}", name=f"rs{mc}")
        nc.vector.reciprocal(rs[:, :], ssum[:, :])
        wt = s_pool.tile([128, 8], F32, tag=f"w{mc}", name=f"w{mc}")
        nc.vector.tensor_scalar_mul(wt[:, :], e[:, :], rs[:, :])
        ds = []
        for k in range(TOPK):
            dg = dg_pool.tile([128, 128], F16, tag="dg", name="dg")
            nc.vector.tensor_scalar_mul(dg[:, :], ident[:, :], wt[:, k : k + 1])
            ds.append(dg)
        return gs, ds

    def wsum(mc, gs, ds):
        pacc = [psa.tile([128, 512], F32, tag="pa", name="pa") for _ in range(4)]
        for k in range(TOPK):
            for nsl in range(4):
                nc.tensor.matmul(
                    pacc[nsl][:, :], ds[k][:, :], gs[k][:, ts(nsl, 512)],
                    start=(k == 0), stop=(k == TOPK - 1),
                )
        for nsl in range(4):
            nc.scalar.activation(acc[mc][:, ts(nsl, 512)], pacc[nsl][:, :], AF.Copy)

    gd = {}
    inv_dft(0)
    gd[0] = topk_gather(0)
    inv_dft(1)
    gd[1] = topk_gather(1)
    wsum(0, *gd[0])
    inv_dft(2)
    gd[2] = topk_gather(2)
    wsum(1, *gd[1])
    inv_dft(3)
    gd[3] = topk_gather(3)
    wsum(2, *gd[2])

    ps5.release()
    po_pool = tc.alloc_tile_pool(name="po", bufs=1, space="PSUM")
    ot_pool = tc.alloc_tile_pool(name="ot", bufs=4, side="right")

    wsum(3, *gd[3])

    # ---- S8: output projection, TRANSPOSED: outT[c, t] = sum_cin Wo[cin, c]
    # * acc[cin, t] + bo[c]. Channel-major output puts the bias on the
    # partition axis (fused into the PSUM copy); the host un-transposes.
    for cb in range(4):
        pss = [po_pool.tile([128, 512], F32, tag=f"po{tb}", name=f"po{tb}")
               for tb in range(4)]
        for kc in range(CN):
            for tb in range(4):
                nc.tensor.matmul(
                    pss[tb][:, :], wo[kc][:, ts(cb, 128)], acc[kc][:, ts(tb, 512)],
                    start=(kc == 0), stop=(kc == CN - 1),
                )
        for tb in range(4):
            ot = ot_pool.tile([128, 512], F16, tag="ot", name="ot")
            nc.scalar.activation(
                ot[:, :], pss[tb][:, :], AF.Identity, bias=bocol[:, cb : cb + 1]
            )
            eng = nc.sync if tb % 2 == 0 else nc.scalar
            eng.dma_start(out_ap[ts(cb, 128), ts(tb, 512)], ot[:, :])

    ot_pool.release()
    po_pool.release()
    psa.release()
    ut_pool.release()
    u_pool.release()
    dg_pool.release()
    g_pool.release()
    s_pool.release()
    z_pool.release()
    ftmp_pool.release()
    fold_pool.release()
    mi_pool.release()
    s_pool0.release()
    acc_pool.release()
    r_pool.release()
    w_pool.release()


def build_module():
    nc = bacc.Bacc(
        "TRN2",
        target_bir_lowering=False,
        debug=False,
        enable_asserts=False,
        num_devices=N_CORES,
        num_swdge_queues=4,
    )
    dr = {}

    def din(name, shape, dt=F32R):
        dr[name] = nc.dram_tensor(name, shape, dt, kind="ExternalInput").ap()

    din("qT", [D, L], F16)
    din("kT", [D, L], F16)
    din("Wq", [128, KC * D], F16)   # tiled: [p, kc*D+j] = W[kc*128+p, j]
    din("Wk", [128, KC * D], F16)
    din("Wo", [128, KC * D], F16)
    din("bqL", [1, D], F16)
    din("bkL2", [1, D], F16)
    din("bqc", [D, 1], F32)
    din("boc", [D, 1], F32)
    din("ident", [128, 128], F16)
    din("B1", [8 * 128, 8 * 128], F16)
    din("B2", [4 * 128, 4 * 128], F16)
    din("B3", [4 * 128, 4 * 128], F16)
    din("IB1", [8 * 128, 1024], F16)
    din("IB2", [4 * 128, 512], F16)
    din("IB3", [4 * 128, 512], F16)
    out_ap = nc.dram_tensor("out", [D, L], F16, kind="ExternalOutput").ap()
    q2 = nc.dram_tensor("q2", [D, 2 * L], F16, kind="Internal").ap()

    with tile.TileContext(nc, trace_sim=False) as tc:
        _kernel_body(tc, dr, out_ap, q2)
    nc.compile()
    return nc


_NC_CACHE = {}


def _tile_w(W):
    return np.ascontiguousarray(
        np.asarray(W, np.float32).reshape(KC, 128, D).transpose(1, 0, 2).reshape(128, KC * D)
    )


def make_in_maps(q, k, Wq, bq, Wk, bk, Wo, bo):
    B1, B2, B3 = _build_dft_mats()

    def tile_fwd(Bm, nch):
        return np.ascontiguousarray(
            Bm.reshape(nch, 128, nch, 128).transpose(2, 1, 0, 3)
            .reshape(nch * 128, nch * 128)
        ).astype(np.float16)

    f32 = np.float32
    shared = {
        "Wq": _tile_w(Wq).astype(np.float16),
        "Wk": _tile_w(Wk).astype(np.float16),
        "Wo": _tile_w(Wo).astype(np.float16),
        "bqL": (np.asarray(bq, f32) * L).reshape(1, D).astype(np.float16),
        "bkL2": (np.asarray(bk, f32) * 2.0).reshape(1, D).astype(np.float16),
        "bqc": np.ascontiguousarray(bq, f32).reshape(D, 1),
        "boc": np.ascontiguousarray(bo, f32).reshape(D, 1),
        "ident": np.eye(128, dtype=np.float16),
        "B1": tile_fwd(B1, 8),
        "B2": tile_fwd(B2, 4),
        "B3": tile_fwd(B3, 4),
        "IB1": np.ascontiguousarray(B1.T).astype(np.float16),
        "IB2": np.ascontiguousarray(B2.T).astype(np.float16),
        "IB3": np.ascontiguousarray(B3.T).astype(np.float16),
    }
    in_maps = []
    for b in range(B):
        m = dict(shared)
        m["qT"] = np.ascontiguousarray(np.asarray(q[b], f32).T).astype(np.float16)
        kt_ = np.asarray(k[b], f32).T
        ko = kt_[:, :1024] - kt_[:, 1024:]
        ke = kt_[:, :1024] + kt_[:, 1024:]
        m["kT"] = np.ascontiguousarray(np.concatenate(
            [ko, ke[:, :512] - ke[:, 512:], ke[:, :512] + ke[:, 512:]], axis=1
        )).astype(np.float16)
        in_maps.append(m)
    return in_maps


def kernel(q, k, v, Wq, bq, Wk, bk, Wv, bv, Wo, bo, _want_results=False,
           _trace=False, **_ignored):
    if "nc" not in _NC_CACHE:
        _NC_CACHE["nc"] = build_module()
    nc = _NC_CACHE["nc"]
    in_maps = make_in_maps(q, k, Wq, bq, Wk, bk, Wo, bo)
    # warmup execution: the first run of a freshly-loaded NEFF on a core that
    # ran a different program can read stale state; run once and discard.
    run_bass_kernel_spmd(nc, in_maps, core_ids=list(range(N_CORES)), trace=False)
    res = run_bass_kernel_spmd(
        nc, in_maps, core_ids=list(range(N_CORES)), trace=_trace
    )
    out = np.stack([np.asarray(res.results[b]["out"], np.float32).T for b in range(B)])
    out = np.ascontiguousarray(out)
    if _want_results:
        return out, res
    return out


if __name__ == "__main__":
    # smoke test with random data
    rng = np.random.default_rng(0)
    q = rng.standard_normal((B, L, D), np.float32)
    k = rng.standard_normal((B, L, D), np.float32)
    s = 1.0 / np.sqrt(D)
    Wq = rng.standard_normal((D, D), np.float32) * s
    Wk = rng.standard_normal((D, D), np.float32) * s
    Wo = rng.standard_normal((D, D), np.float32) * s
    z = np.zeros(D, np.float32)
    out = kernel(q, k, None, Wq, z, Wk, z, None, None, Wo, z)
    print("out", out.shape, out.dtype, float(np.abs(out).sum()))

